# revision 10
# baseline (speedup 1.0000x reference)
"""Multi-head attention on 8 Trainium2 cores — v2b.

Sharding: core c handles batch b = c // 4 and a quad of 4 heads
(hq = c % 4 -> heads 4*hq .. 4*hq+3) as two head-pairs of 64+64 = 128
partitions. w_q/w_k/w_v split column-wise by head, w_out row-wise;
per-batch partial outputs are summed on host.

v2b structure (vs v1):
  - scores for the even/odd head of a pair issue back-to-back as
    64x128 PE row tiles (0,0)/(64,0) -> they execute concurrently on
    disjoint row groups + separate PSUM banks (2x effective scores).
  - qT/kT/vT fully resident in SBUF; all input DMA is issued up front
    on the sync/gpsimd/vector queues.  The scalar (ACT) queue carries
    only the small weight tensors so exp never queues behind DMA.
  - the qs=0 score steps + exp + C interleave INTO the projection
    phase (kh/vh production) so the ACT engine starts ~14us in
    instead of ~70us.
  - PSUM: scores 2x[128,1024] + C 2x[65,512] + shared proj/D
    2x[128,512] = exactly 8 banks.  Proj/D psum groups are emitted
    atomically so the shared pool never deadlocks a long-lived
    accumulation against fillers.
"""

import numpy as np

B = 2
S = 2048
D = 1024
NH = 16
DH = 64
HEADS_PER_CORE = 4
N_CORES = 8

_NC = None


def _build():
    import concourse.bacc as bacc
    import concourse.tile as tile
    import concourse.mybir as mybir

    fp32 = mybir.dt.float32
    bf16 = mybir.dt.bfloat16
    add = mybir.AluOpType.add
    mult = mybir.AluOpType.mult
    Exp = mybir.ActivationFunctionType.Exp

    nc = bacc.Bacc("TRN2", target_bir_lowering=False)

    # all inputs host-packed to the exact SBUF layout so each is a
    # few large fully-contiguous DMA transfers:
    #   x: [128, ns*4096 + kc*512 + c]  (ns-major blocks of 1MB)
    #   w: [128, kc*256 + c]            (single 512KB transfer)
    qT = nc.dram_tensor("qT", (128, 16384), bf16, kind="ExternalInput")
    kT = nc.dram_tensor("kT", (128, 16384), bf16, kind="ExternalInput")
    vT = nc.dram_tensor("vT", (128, 16384), bf16, kind="ExternalInput")
    wq = nc.dram_tensor("wq", (128, 2048), bf16, kind="ExternalInput")
    wk = nc.dram_tensor("wk", (128, 2048), bf16, kind="ExternalInput")
    wv = nc.dram_tensor("wv", (128, 2048), bf16, kind="ExternalInput")
    wo = nc.dram_tensor("wo", (128, 2048), bf16, kind="ExternalInput")
    bq = nc.dram_tensor("bq", (128, 2), fp32, kind="ExternalInput")
    bk = nc.dram_tensor("bk", (128, 2), fp32, kind="ExternalInput")
    bv = nc.dram_tensor("bv", (1, 256), fp32, kind="ExternalInput")
    out = nc.dram_tensor("out", (S, D), fp32, kind="ExternalOutput")

    with tile.TileContext(nc) as tc:
        with tc.tile_pool(name="persist", bufs=1) as P:
            qhT = [P.tile((128, S), bf16, name=f"qhT{p}") for p in range(2)]
            khT = [P.tile((128, S), bf16, name=f"khT{p}") for p in range(2)]
            vh = [P.tile((128, 16 * 65), bf16, name=f"vh{h}") for h in range(4)]
            outnT = [P.tile((128, S), bf16, name=f"outnT{p}") for p in range(2)]
            wq_sb = P.tile((128, 8 * 256), bf16, name="wq_sb")
            wk_sb = P.tile((128, 8 * 256), bf16, name="wk_sb")
            wv_sb = P.tile((128, 8 * 256), bf16, name="wv_sb")
            wo_t = P.tile((128, 2048), bf16, name="wo_t")
            wo_sb = [wo_t[:, 0:1024], wo_t[:, 1024:2048]]
            bq_sb = P.tile((128, 2), fp32, name="bq_sb")
            bk_sb = P.tile((128, 2), fp32, name="bk_sb")
            bv_row = P.tile((1, 256), fp32, name="bv_row")
            bv_bc = P.tile((128, 256), fp32, name="bv_bc")
            qxt = P.tile((128, 16384), bf16, name="qxt")
            kxt = P.tile((128, 16384), bf16, name="kxt")
            vxt = P.tile((128, 16384), bf16, name="vxt")

            # ones columns for the denominator trick: memset the whole vh
            # tiles to 1.0 on gpsimd (proj overwrites the 64 data columns)
            for h in range(4):
                nc.gpsimd.memset(vh[h][:], 1.0)

            # ---------------- DMA schedule ----------------
            # weights lead the sync queue so they complete before the bulk
            # x blocks behind them; scalar (ACT) queue carries only the two
            # tiny bias transfers and is immediately free for exp.
            nc.sync.dma_start(wq_sb[:], wq[:])
            nc.sync.dma_start(wk_sb[:], wk[:])
            nc.scalar.dma_start(bq_sb[:], bq[:])
            nc.scalar.dma_start(bk_sb[:], bk[:])

            def xblk(eng, dst, srcd, ns):
                eng.dma_start(
                    dst[:, ns * 4096:(ns + 1) * 4096],
                    srcd[:, ns * 4096:(ns + 1) * 4096],
                )

            # need-ordered x in column-half blocks: each ns-block split
            # across both bulk queues so the two halves arrive in parallel.
            def xhalf(eng, dst, srcd, ns, half):
                c0 = ns * 4096 + half * 2048
                eng.dma_start(dst[:, c0:c0 + 2048], srcd[:, c0:c0 + 2048])

            def xpair(dst, srcd, ns):
                xhalf(nc.sync, dst, srcd, ns, 0)
                xhalf(nc.gpsimd, dst, srcd, ns, 1)

            # gpsimd leads with q0/k0 halves while sync still moves weights
            xhalf(nc.gpsimd, qxt, qT, 0, 1)
            xhalf(nc.gpsimd, kxt, kT, 0, 1)
            xhalf(nc.sync, qxt, qT, 0, 0)
            xhalf(nc.sync, kxt, kT, 0, 0)
            xpair(kxt, kT, 1)
            xpair(qxt, qT, 1)
            xpair(kxt, kT, 2)
            nc.gpsimd.dma_start(wv_sb[:], wv[:])
            nc.gpsimd.dma_start(bv_row[:], bv[:])
            nc.gpsimd.partition_broadcast(bv_bc[:], bv_row[:])
            xpair(kxt, kT, 3)
            xpair(vxt, vT, 0)
            xpair(vxt, vT, 1)
            xpair(vxt, vT, 2)
            xpair(vxt, vT, 3)
            xpair(qxt, qT, 2)
            xpair(qxt, qT, 3)
            nc.gpsimd.dma_start(wo_t[:], wo[:])

            # ---------------- compute ----------------
            with tc.tile_pool(name="psS", bufs=2, space="PSUM") as PS, \
                 tc.tile_pool(name="psC", bufs=2, space="PSUM") as PC, \
                 tc.tile_pool(name="psDF", bufs=2, space="PSUM") as PD, \
                 tc.tile_pool(name="expP", bufs=16) as EP, \
                 tc.tile_pool(name="nrm", bufs=2) as NP, \
                 tc.tile_pool(name="outP", bufs=2) as OP:

                # one projection group: 8 accum matmuls + bias add.
                # inject() runs mid-group so a score step can slip in
                # (scores use the PS pool, no PD hazard) and the ACT
                # engine is not starved by long proj bursts.
                def proj_group(xt, w_sb, b_sb, dstT, ns, p, inject=None):
                    ps = PD.tile((128, 512), fp32, name="pps", tag="dps")
                    for kc in range(8):
                        if kc == 4 and inject is not None:
                            inject()
                        nc.tensor.matmul(
                            ps[:],
                            w_sb[:, kc * 256 + p * 128:kc * 256 + (p + 1) * 128],
                            xt[:, ns * 4096 + kc * 512:ns * 4096 + (kc + 1) * 512],
                            start=(kc == 0),
                            stop=(kc == 7),
                        )
                    nc.vector.tensor_scalar_add(
                        dstT[p][:, ns * 512:(ns + 1) * 512], ps[:], b_sb[:, p:p + 1]
                    )

                # one vh group: j-chunk jc for all 4 heads
                def vh_group(jc, inject=None):
                    vh_done[0] = jc + 1
                    ps = PD.tile((128, 512), fp32, name="vps", tag="dps")
                    for kc in range(8):
                        if kc == 4 and inject is not None:
                            inject()
                        nc.tensor.matmul(
                            ps[:, 0:256],
                            vxt[:, (jc // 4) * 4096 + kc * 512 + (jc % 4) * 128:
                                 (jc // 4) * 4096 + kc * 512 + (jc % 4) * 128 + 128],
                            wv_sb[:, kc * 256:(kc + 1) * 256],
                            start=(kc == 0),
                            stop=(kc == 7),
                        )
                    for h in range(4):
                        nc.vector.scalar_tensor_tensor(
                            vh[h][:, jc * 65:jc * 65 + 64],
                            ps[:, h * 64:(h + 1) * 64],
                            1.0,
                            bv_bc[:, h * 64:(h + 1) * 64],
                            op0=mult,
                            op1=add,
                        )

                # ----- scores / exp / C streams -----
                cq = []
                cps_cur = {}
                norm_done = [0] * 4

                act_free = [False]
                Copy = mybir.ActivationFunctionType.Copy

                def normalize(ph, pqs, pcps):
                    p, off = ph // 2, (ph % 2) * 64
                    den = NP.tile((1, 512), fp32, name="den")
                    if act_free[0]:
                        nc.scalar.activation(den[:], pcps[64:65, :], Copy, bias=0.0)
                    else:
                        nc.vector.tensor_scalar_add(den[:], pcps[64:65, :], 0.0)
                    rec = NP.tile((1, 512), fp32, name="rec")
                    nc.vector.reciprocal_approx_fast(rec[:], den[:])
                    rbc = NP.tile((64, 512), fp32, name="rbc")
                    nc.gpsimd.partition_broadcast(rbc[:], rec[:])
                    nc.vector.scalar_tensor_tensor(
                        outnT[p][off:off + 64, pqs * 512:(pqs + 1) * 512],
                        pcps[0:64, :],
                        1.0,
                        rbc[:],
                        op0=mult,
                        op1=mult,
                    )

                def score_step(qs, p, jp):
                    if jp == 0:
                        cps_cur[p] = (
                            PC.tile((65, 512), fp32, name="cps_e", tag="cps"),
                            PC.tile((65, 512), fp32, name="cps_o", tag="cps"),
                        )
                    sps_e = PS.tile((128, 1024), fp32, name="sps_e", tag="sps")
                    sps_o = PS.tile((128, 1024), fp32, name="sps_o", tag="sps")
                    for half in range(2):
                        jc = jp * 2 + half
                        for off, sps in ((0, sps_e), (64, sps_o)):
                            nc.tensor.matmul(
                                sps[:, half * 512:(half + 1) * 512],
                                khT[p][off:off + 64, jc * 128:(jc + 1) * 128],
                                qhT[p][off:off + 64, qs * 512:(qs + 1) * 512],
                                start=True,
                                stop=True,
                            )
                    ex_e = EP.tile((128, 1024), bf16, name="ex_e", tag="ex")
                    nc.scalar.activation(ex_e[:], sps_e[:], Exp, bias=0.0, scale=0.125)
                    ex_o = EP.tile((128, 1024), bf16, name="ex_o", tag="ex")
                    nc.scalar.activation(ex_o[:], sps_o[:], Exp, bias=0.0, scale=0.125)
                    ce, co = cps_cur[p]
                    cq.append((2 * p, qs, jp, ce, ex_e))
                    cq.append((2 * p + 1, qs, jp, co, ex_o))

                S_steps = iter(
                    [(qs, p, jp) for qs in range(4) for p in range(2) for jp in range(8)]
                )

                def run_s(n=1):
                    for _ in range(n):
                        nxt = next(S_steps, None)
                        if nxt is not None:
                            score_step(*nxt)

                def c_entry():
                    ph, pqs, pjp, pcps, pex = cq.pop(0)
                    for half in range(2):
                        pjc = pjp * 2 + half
                        nc.tensor.matmul(
                            pcps[:],
                            vh[ph][:, pjc * 65:(pjc + 1) * 65],
                            pex[:, half * 512:(half + 1) * 512],
                            start=(pjc == 0),
                            stop=(pjc == 15),
                        )
                    if pjp == 7:
                        normalize(ph, pqs, pcps)
                        norm_done[pqs] += 1

                vh_done = [0]  # j-chunks of vh emitted so far

                def run_c(n=1):
                    for _ in range(n):
                        if cq and 2 * cq[0][2] + 1 < vh_done[0]:
                            c_entry()

                # D stage generator: yields once per psum group
                def d_qs(qs):
                    for qq in range(4):
                        qc0 = qs * 512 + qq * 128
                        osb = OP.tile((128, D), fp32, name="osb")
                        for nsd in range(2):
                            dps = PD.tile((128, 512), fp32, name="dps", tag="dps")
                            for p in range(2):
                                nc.tensor.matmul(
                                    dps[:],
                                    outnT[p][:, qc0:qc0 + 128],
                                    wo_sb[p][:, nsd * 512:(nsd + 1) * 512],
                                    start=(p == 0),
                                    stop=(p == 1),
                                )
                            if act_free[0]:
                                nc.scalar.activation(
                                    osb[:, nsd * 512:(nsd + 1) * 512], dps[:],
                                    Copy, bias=0.0,
                                )
                            else:
                                nc.vector.tensor_scalar_add(
                                    osb[:, nsd * 512:(nsd + 1) * 512], dps[:], 0.0
                                )
                            yield
                        nc.sync.dma_start(out[qc0:qc0 + 64, :], osb[0:64, :])
                        nc.gpsimd.dma_start(out[qc0 + 64:qc0 + 128, :], osb[64:128, :])

                D_iters = [d_qs(0), d_qs(1), d_qs(2), d_qs(3)]
                _F_done = object()
                F_qh = iter([(2, 0), (2, 1), (3, 0), (3, 1)])

                def run_f(n=1):
                    for _ in range(n):
                        nxt = next(F_qh, None)
                        if nxt is not None:
                            proj_group(qxt, wq_sb, bq_sb, qhT, *nxt)
                            continue
                        for qs2 in range(4):
                            if norm_done[qs2] == 4 and D_iters[qs2] is not None:
                                if next(D_iters[qs2], _F_done) is _F_done:
                                    D_iters[qs2] = None
                                    continue
                                break

                # ---------------- emission program ----------------
                # Just-in-time scheduler: keep ACT fed with score steps,
                # drain C close behind, pace proj/vh groups by their DMA
                # arrival ETAs so a stalled matmul never head-of-line
                # blocks ready work on the in-order PE queue.
                MM = 250.0
                clock = {"pe": 10000.0, "act": 14000.0}

                K_ETA = [14500, 18000, 21500, 26500]
                Q_ETA = [14500, 21500, 41000, 44000]
                V_ETA = [29000, 31500, 34000, 36500]

                P_order = (
                    [("q", 0, 0), ("q", 0, 1)]
                    + [("k", ns, p) for ns in range(4) for p in range(2)]
                    + [("q", 1, 0), ("q", 1, 1)]
                    + [("v", jc) for jc in range(16)]
                    + [("q", 2, 0), ("q", 2, 1), ("q", 3, 0), ("q", 3, 1)]
                )
                P_idx = [0]
                qh_done = set()
                kh_ns_done = [0, 0]

                def p_eta():
                    g = P_order[P_idx[0]]
                    if g[0] == "k":
                        return K_ETA[g[1]]
                    if g[0] == "q":
                        return Q_ETA[g[1]]
                    return V_ETA[g[1] // 4]

                def maybe_s():
                    if s_ready() and clock["act"] - clock["pe"] < 2400:
                        emit_s()

                def emit_p():
                    g = P_order[P_idx[0]]
                    P_idx[0] += 1
                    if g[0] == "q":
                        proj_group(qxt, wq_sb, bq_sb, qhT, g[1], g[2], maybe_s)
                        qh_done.add((g[1], g[2]))
                    elif g[0] == "k":
                        proj_group(kxt, wk_sb, bk_sb, khT, g[1], g[2], maybe_s)
                        if g[2] == 1:
                            kh_ns_done[0] = g[1] + 1
                            kh_ns_done[1] = g[1] + 1
                    else:
                        vh_group(g[1], maybe_s)
                    clock["pe"] += 8 * MM

                S_list = [(qs, p, jp) for qs in range(4) for p in range(2)
                          for jp in range(8)]
                S_idx = [0]

                def s_ready():
                    if S_idx[0] >= len(S_list):
                        return False
                    qs, p, jp = S_list[S_idx[0]]
                    return (qs, p) in qh_done and kh_ns_done[p] * 4 >= 2 * jp + 2

                def emit_s():
                    score_step(*S_list[S_idx[0]])
                    S_idx[0] += 1
                    if S_idx[0] >= len(S_list):
                        act_free[0] = True
                    clock["pe"] += 4 * MM
                    clock["act"] = max(clock["act"], clock["pe"]) + 2292

                def c_ready():
                    return bool(cq) and 2 * cq[0][2] + 1 < vh_done[0]

                def emit_c():
                    c_entry()
                    clock["pe"] += 2 * MM

                def emit_d():
                    for qs2 in range(4):
                        if norm_done[qs2] == 4 and D_iters[qs2] is not None:
                            if next(D_iters[qs2], _F_done) is _F_done:
                                D_iters[qs2] = None
                                continue
                            clock["pe"] += 2 * MM
                            return True
                    return False

                def d_left():
                    return any(it is not None for it in D_iters)

                while (S_idx[0] < len(S_list) or cq or P_idx[0] < len(P_order)
                       or d_left()):
                    act_lead = clock["act"] - clock["pe"]
                    if s_ready() and act_lead < 4600:
                        emit_s()
                        continue
                    c_floor = 2 if S_idx[0] < 56 else 0
                    if c_ready() and len(cq) > c_floor:
                        emit_c()
                        continue
                    if P_idx[0] < len(P_order) and p_eta() < clock["pe"] + 1200:
                        emit_p()
                        continue
                    if c_ready():
                        emit_c()
                        continue
                    if emit_d():
                        continue
                    if s_ready():
                        emit_s()
                        continue
                    if P_idx[0] < len(P_order):
                        emit_p()
                        continue
                    if cq:
                        c_entry()
                        continue
                    break

    nc.compile()
    return nc


def _get_nc():
    global _NC
    if _NC is None:
        _NC = _build()
    return _NC


def run(inputs, trace=False, trace_cores=None):
    from concourse.bass_utils import run_bass_kernel_spmd

    q = np.asarray(inputs["q"], np.float32)
    k = np.asarray(inputs["k"], np.float32)
    v = np.asarray(inputs["v"], np.float32)
    w_q = np.asarray(inputs["w_q"], np.float32)
    w_k = np.asarray(inputs["w_k"], np.float32)
    w_v = np.asarray(inputs["w_v"], np.float32)
    w_out = np.asarray(inputs["w_out"], np.float32)
    b_q = np.asarray(inputs["b_q"], np.float32)
    b_k = np.asarray(inputs["b_k"], np.float32)
    b_v = np.asarray(inputs["b_v"], np.float32)
    b_out = np.asarray(inputs["b_out"], np.float32)

    import ml_dtypes
    bf16 = ml_dtypes.bfloat16

    def pack_x(x):
        # [S, D] -> transposed, SBUF-layout packed [128, ns*4096+kc*512+c]
        xT = x.T.reshape(8, 128, 4, 512)
        return np.ascontiguousarray(
            xT.transpose(1, 2, 0, 3).reshape(128, 16384).astype(bf16))

    def pack_w(w):
        # [1024, 256] (pre-transposed slice) -> [128, kc*256+c]
        return np.ascontiguousarray(
            w.reshape(8, 128, 256).transpose(1, 0, 2).reshape(128, 2048).astype(bf16))

    xT = {b: {} for b in range(B)}
    for b in range(B):
        xT[b]["qT"] = pack_x(q[b])
        xT[b]["kT"] = pack_x(k[b])
        xT[b]["vT"] = pack_x(v[b])

    in_maps = []
    for c in range(N_CORES):
        b, hq = c // 4, c % 4
        rows = slice(hq * 256, (hq + 1) * 256)
        wo_p = w_out[:, rows].T.reshape(2, 128, 1024)
        in_maps.append({
            "qT": xT[b]["qT"],
            "kT": xT[b]["kT"],
            "vT": xT[b]["vT"],
            "wq": pack_w(w_q[rows, :].T),
            "wk": pack_w(w_k[rows, :].T),
            "wv": pack_w(w_v[rows, :].T),
            "wo": np.ascontiguousarray(
                wo_p.transpose(1, 0, 2).reshape(128, 2048).astype(bf16)),
            "bq": np.ascontiguousarray(b_q[rows].reshape(2, 128).T),
            "bk": np.ascontiguousarray(b_k[rows].reshape(2, 128).T),
            "bv": np.ascontiguousarray(b_v[rows].reshape(1, 256)),
        })

    nc = _get_nc()
    res = run_bass_kernel_spmd(
        nc, in_maps, core_ids=list(range(N_CORES)), trace=trace,
        trace_cores=trace_cores,
    )
    full = np.zeros((B, S, D), np.float32)
    for c in range(N_CORES):
        full[c // 4] += np.asarray(res.results[c]["out"])
    full += b_out.reshape(1, 1, D)
    return full, res.exec_time_ns


def kernel(**inputs):
    return run(inputs, trace=False)[0]


# revision 11
# speedup vs baseline: 1.0287x; 1.0287x over previous
"""Multi-head attention on 8 Trainium2 cores — v2b.

Sharding: core c handles batch b = c // 4 and a quad of 4 heads
(hq = c % 4 -> heads 4*hq .. 4*hq+3) as two head-pairs of 64+64 = 128
partitions. w_q/w_k/w_v split column-wise by head, w_out row-wise;
per-batch partial outputs are summed on host.

v2b structure (vs v1):
  - scores for the even/odd head of a pair issue back-to-back as
    64x128 PE row tiles (0,0)/(64,0) -> they execute concurrently on
    disjoint row groups + separate PSUM banks (2x effective scores).
  - qT/kT/vT fully resident in SBUF; all input DMA is issued up front
    on the sync/gpsimd/vector queues.  The scalar (ACT) queue carries
    only the small weight tensors so exp never queues behind DMA.
  - the qs=0 score steps + exp + C interleave INTO the projection
    phase (kh/vh production) so the ACT engine starts ~14us in
    instead of ~70us.
  - PSUM: scores 2x[128,1024] + C 2x[65,512] + shared proj/D
    2x[128,512] = exactly 8 banks.  Proj/D psum groups are emitted
    atomically so the shared pool never deadlocks a long-lived
    accumulation against fillers.
"""

import numpy as np

B = 2
S = 2048
D = 1024
NH = 16
DH = 64
HEADS_PER_CORE = 4
N_CORES = 8

_NC = None


def _build():
    import concourse.bacc as bacc
    import concourse.tile as tile
    import concourse.mybir as mybir

    fp32 = mybir.dt.float32
    bf16 = mybir.dt.bfloat16
    add = mybir.AluOpType.add
    mult = mybir.AluOpType.mult
    Exp = mybir.ActivationFunctionType.Exp

    nc = bacc.Bacc("TRN2", target_bir_lowering=False)

    # all inputs host-packed to the exact SBUF layout so each is a
    # few large fully-contiguous DMA transfers:
    #   x: [128, ns*4096 + kc*512 + c]  (ns-major blocks of 1MB)
    #   w: [128, kc*256 + c]            (single 512KB transfer)
    qT = nc.dram_tensor("qT", (128, 16384), bf16, kind="ExternalInput")
    kT = nc.dram_tensor("kT", (128, 16384), bf16, kind="ExternalInput")
    vT = nc.dram_tensor("vT", (128, 16384), bf16, kind="ExternalInput")
    wq = nc.dram_tensor("wq", (128, 2048), bf16, kind="ExternalInput")
    wk = nc.dram_tensor("wk", (128, 2048), bf16, kind="ExternalInput")
    wv = nc.dram_tensor("wv", (128, 2048), bf16, kind="ExternalInput")
    wo = nc.dram_tensor("wo", (128, 2048), bf16, kind="ExternalInput")
    bq = nc.dram_tensor("bq", (128, 2), fp32, kind="ExternalInput")
    bk = nc.dram_tensor("bk", (128, 2), fp32, kind="ExternalInput")
    bv = nc.dram_tensor("bv", (1, 256), fp32, kind="ExternalInput")
    out = nc.dram_tensor("out", (S, D), fp32, kind="ExternalOutput")

    with tile.TileContext(nc) as tc:
        with tc.tile_pool(name="persist", bufs=1) as P:
            qhT = [P.tile((128, S), bf16, name=f"qhT{p}") for p in range(2)]
            khT = [P.tile((128, S), bf16, name=f"khT{p}") for p in range(2)]
            vh = [P.tile((128, 16 * 65), bf16, name=f"vh{h}") for h in range(4)]
            outnT = [P.tile((128, S), bf16, name=f"outnT{p}") for p in range(2)]
            wq_sb = P.tile((128, 8 * 256), bf16, name="wq_sb")
            wk_sb = P.tile((128, 8 * 256), bf16, name="wk_sb")
            wv_sb = P.tile((128, 8 * 256), bf16, name="wv_sb")
            wo_t = P.tile((128, 2048), bf16, name="wo_t")
            wo_sb = [wo_t[:, 0:1024], wo_t[:, 1024:2048]]
            bq_sb = P.tile((128, 2), fp32, name="bq_sb")
            bk_sb = P.tile((128, 2), fp32, name="bk_sb")
            bv_row = P.tile((1, 256), fp32, name="bv_row")
            bv_bc = P.tile((128, 256), fp32, name="bv_bc")
            qxt = P.tile((128, 16384), bf16, name="qxt")
            kxt = P.tile((128, 16384), bf16, name="kxt")
            vxt = P.tile((128, 16384), bf16, name="vxt")

            # ones columns for the denominator trick: memset the whole vh
            # tiles to 1.0 on gpsimd (proj overwrites the 64 data columns)
            for h in range(4):
                nc.gpsimd.memset(vh[h][:], 1.0)

            # ---------------- DMA schedule ----------------
            # weights lead the sync queue so they complete before the bulk
            # x blocks behind them; scalar (ACT) queue carries only the two
            # tiny bias transfers and is immediately free for exp.
            nc.sync.dma_start(wq_sb[:], wq[:])
            nc.sync.dma_start(wk_sb[:], wk[:])
            nc.scalar.dma_start(bq_sb[:], bq[:])
            nc.scalar.dma_start(bk_sb[:], bk[:])

            def xblk(eng, dst, srcd, ns):
                eng.dma_start(
                    dst[:, ns * 4096:(ns + 1) * 4096],
                    srcd[:, ns * 4096:(ns + 1) * 4096],
                )

            # need-ordered x in column-half blocks: each ns-block split
            # across both bulk queues so the two halves arrive in parallel.
            def xhalf(eng, dst, srcd, ns, half):
                c0 = ns * 4096 + half * 2048
                eng.dma_start(dst[:, c0:c0 + 2048], srcd[:, c0:c0 + 2048])

            def xpair(dst, srcd, ns):
                xhalf(nc.sync, dst, srcd, ns, 0)
                xhalf(nc.gpsimd, dst, srcd, ns, 1)

            # gpsimd leads with q0/k0 halves while sync still moves weights
            xhalf(nc.gpsimd, qxt, qT, 0, 1)
            xhalf(nc.gpsimd, kxt, kT, 0, 1)
            xhalf(nc.sync, qxt, qT, 0, 0)
            xhalf(nc.sync, kxt, kT, 0, 0)
            xpair(kxt, kT, 1)
            xpair(qxt, qT, 1)
            xpair(kxt, kT, 2)
            nc.gpsimd.dma_start(wv_sb[:], wv[:])
            nc.gpsimd.dma_start(bv_row[:], bv[:])
            nc.gpsimd.partition_broadcast(bv_bc[:], bv_row[:])
            xpair(kxt, kT, 3)
            xpair(vxt, vT, 0)
            xpair(vxt, vT, 1)
            xpair(vxt, vT, 2)
            xpair(vxt, vT, 3)
            xpair(qxt, qT, 2)
            xpair(qxt, qT, 3)
            nc.gpsimd.dma_start(wo_t[:], wo[:])

            # ---------------- compute ----------------
            with tc.tile_pool(name="psS", bufs=2, space="PSUM") as PS, \
                 tc.tile_pool(name="psC", bufs=2, space="PSUM") as PC, \
                 tc.tile_pool(name="psDF", bufs=2, space="PSUM") as PD, \
                 tc.tile_pool(name="expP", bufs=16) as EP, \
                 tc.tile_pool(name="nrm", bufs=2) as NP, \
                 tc.tile_pool(name="outP", bufs=2) as OP:

                # one projection group: 8 accum matmuls + bias add.
                # inject() runs mid-group so a score step can slip in
                # (scores use the PS pool, no PD hazard) and the ACT
                # engine is not starved by long proj bursts.
                def proj_group(xt, w_sb, b_sb, dstT, ns, p, inject=None):
                    ps = PD.tile((128, 512), fp32, name="pps", tag="dps")
                    for kc in range(8):
                        if kc == 4 and inject is not None:
                            inject()
                        nc.tensor.matmul(
                            ps[:],
                            w_sb[:, kc * 256 + p * 128:kc * 256 + (p + 1) * 128],
                            xt[:, ns * 4096 + kc * 512:ns * 4096 + (kc + 1) * 512],
                            start=(kc == 0),
                            stop=(kc == 7),
                        )
                    nc.vector.tensor_scalar_add(
                        dstT[p][:, ns * 512:(ns + 1) * 512], ps[:], b_sb[:, p:p + 1]
                    )

                # one vh group: j-chunk jc for all 4 heads
                def vh_group(jc, inject=None):
                    vh_done[0] = jc + 1
                    ps = PD.tile((128, 512), fp32, name="vps", tag="dps")
                    for kc in range(8):
                        if kc == 4 and inject is not None:
                            inject()
                        nc.tensor.matmul(
                            ps[:, 0:256],
                            vxt[:, (jc // 4) * 4096 + kc * 512 + (jc % 4) * 128:
                                 (jc // 4) * 4096 + kc * 512 + (jc % 4) * 128 + 128],
                            wv_sb[:, kc * 256:(kc + 1) * 256],
                            start=(kc == 0),
                            stop=(kc == 7),
                        )
                    for h in range(4):
                        nc.vector.scalar_tensor_tensor(
                            vh[h][:, jc * 65:jc * 65 + 64],
                            ps[:, h * 64:(h + 1) * 64],
                            1.0,
                            bv_bc[:, h * 64:(h + 1) * 64],
                            op0=mult,
                            op1=add,
                        )

                # ----- scores / exp / C streams -----
                cq = []
                cps_cur = {}
                norm_done = [0] * 4

                act_free = [False]
                Copy = mybir.ActivationFunctionType.Copy

                def normalize(ph, pqs, pcps):
                    p, off = ph // 2, (ph % 2) * 64
                    den = NP.tile((1, 512), fp32, name="den")
                    if act_free[0]:
                        nc.scalar.activation(den[:], pcps[64:65, :], Copy, bias=0.0)
                    else:
                        nc.vector.tensor_scalar_add(den[:], pcps[64:65, :], 0.0)
                    rec = NP.tile((1, 512), fp32, name="rec")
                    nc.vector.reciprocal_approx_fast(rec[:], den[:])
                    rbc = NP.tile((64, 512), fp32, name="rbc")
                    nc.gpsimd.partition_broadcast(rbc[:], rec[:])
                    nc.vector.scalar_tensor_tensor(
                        outnT[p][off:off + 64, pqs * 512:(pqs + 1) * 512],
                        pcps[0:64, :],
                        1.0,
                        rbc[:],
                        op0=mult,
                        op1=mult,
                    )

                def score_step(qs, p, jp):
                    if jp == 0:
                        cps_cur[p] = (
                            PC.tile((65, 512), fp32, name="cps_e", tag="cps"),
                            PC.tile((65, 512), fp32, name="cps_o", tag="cps"),
                        )
                    sps_e = PS.tile((128, 1024), fp32, name="sps_e", tag="sps")
                    sps_o = PS.tile((128, 1024), fp32, name="sps_o", tag="sps")
                    for half in range(2):
                        jc = jp * 2 + half
                        for off, sps in ((0, sps_e), (64, sps_o)):
                            nc.tensor.matmul(
                                sps[:, half * 512:(half + 1) * 512],
                                khT[p][off:off + 64, jc * 128:(jc + 1) * 128],
                                qhT[p][off:off + 64, qs * 512:(qs + 1) * 512],
                                start=True,
                                stop=True,
                            )
                    ex_e = EP.tile((128, 1024), bf16, name="ex_e", tag="ex")
                    nc.scalar.activation(ex_e[:], sps_e[:], Exp, bias=0.0, scale=0.125)
                    ex_o = EP.tile((128, 1024), bf16, name="ex_o", tag="ex")
                    nc.scalar.activation(ex_o[:], sps_o[:], Exp, bias=0.0, scale=0.125)
                    ce, co = cps_cur[p]
                    cq.append((2 * p, qs, jp, ce, ex_e))
                    cq.append((2 * p + 1, qs, jp, co, ex_o))

                S_steps = iter(
                    [(qs, p, jp) for qs in range(4) for p in range(2) for jp in range(8)]
                )

                def run_s(n=1):
                    for _ in range(n):
                        nxt = next(S_steps, None)
                        if nxt is not None:
                            score_step(*nxt)

                def c_entry():
                    ph, pqs, pjp, pcps, pex = cq.pop(0)
                    for half in range(2):
                        pjc = pjp * 2 + half
                        nc.tensor.matmul(
                            pcps[:],
                            vh[ph][:, pjc * 65:(pjc + 1) * 65],
                            pex[:, half * 512:(half + 1) * 512],
                            start=(pjc == 0),
                            stop=(pjc == 15),
                        )
                    if pjp == 7:
                        normalize(ph, pqs, pcps)
                        norm_done[pqs] += 1

                vh_done = [0]  # j-chunks of vh emitted so far

                def run_c(n=1):
                    for _ in range(n):
                        if cq and 2 * cq[0][2] + 1 < vh_done[0]:
                            c_entry()

                # D stage generator: yields once per psum group
                def d_qs(qs):
                    for qq in range(4):
                        qc0 = qs * 512 + qq * 128
                        osb = OP.tile((128, D), fp32, name="osb")
                        for nsd in range(2):
                            dps = PD.tile((128, 512), fp32, name="dps", tag="dps")
                            for p in range(2):
                                nc.tensor.matmul(
                                    dps[:],
                                    outnT[p][:, qc0:qc0 + 128],
                                    wo_sb[p][:, nsd * 512:(nsd + 1) * 512],
                                    start=(p == 0),
                                    stop=(p == 1),
                                )
                            if act_free[0]:
                                nc.scalar.activation(
                                    osb[:, nsd * 512:(nsd + 1) * 512], dps[:],
                                    Copy, bias=0.0,
                                )
                            else:
                                nc.vector.tensor_scalar_add(
                                    osb[:, nsd * 512:(nsd + 1) * 512], dps[:], 0.0
                                )
                            yield
                        nc.sync.dma_start(out[qc0:qc0 + 64, :], osb[0:64, :])
                        nc.gpsimd.dma_start(out[qc0 + 64:qc0 + 128, :], osb[64:128, :])

                D_iters = [d_qs(0), d_qs(1), d_qs(2), d_qs(3)]
                _F_done = object()
                F_qh = iter([(2, 0), (2, 1), (3, 0), (3, 1)])

                def run_f(n=1):
                    for _ in range(n):
                        nxt = next(F_qh, None)
                        if nxt is not None:
                            proj_group(qxt, wq_sb, bq_sb, qhT, *nxt)
                            continue
                        for qs2 in range(4):
                            if norm_done[qs2] == 4 and D_iters[qs2] is not None:
                                if next(D_iters[qs2], _F_done) is _F_done:
                                    D_iters[qs2] = None
                                    continue
                                break

                # ---------------- emission program ----------------
                # Just-in-time scheduler: keep ACT fed with score steps,
                # drain C close behind, pace proj/vh groups by their DMA
                # arrival ETAs so a stalled matmul never head-of-line
                # blocks ready work on the in-order PE queue.
                MM = 250.0
                clock = {"pe": 10000.0, "act": 14000.0}

                K_ETA = [14500, 18000, 21500, 26500]
                Q_ETA = [14500, 21500, 41000, 44000]
                V_ETA = [29000, 31500, 34000, 36500]

                P_order = (
                    [("q", 0, 0), ("q", 0, 1)]
                    + [("k", ns, p) for ns in range(4) for p in range(2)]
                    + [("q", 1, 0), ("q", 1, 1)]
                    + [("v", jc) for jc in range(16)]
                    + [("q", 2, 0), ("q", 2, 1), ("q", 3, 0), ("q", 3, 1)]
                )
                P_idx = [0]
                qh_done = set()
                kh_ns_done = [0, 0]

                def p_eta():
                    g = P_order[P_idx[0]]
                    if g[0] == "k":
                        return K_ETA[g[1]]
                    if g[0] == "q":
                        return Q_ETA[g[1]]
                    return V_ETA[g[1] // 4]

                def maybe_s():
                    if s_ready() and clock["act"] - clock["pe"] < 2400:
                        emit_s()

                def emit_p():
                    g = P_order[P_idx[0]]
                    P_idx[0] += 1
                    if g[0] == "q":
                        proj_group(qxt, wq_sb, bq_sb, qhT, g[1], g[2])
                        qh_done.add((g[1], g[2]))
                    elif g[0] == "k":
                        proj_group(kxt, wk_sb, bk_sb, khT, g[1], g[2])
                        if g[2] == 1:
                            kh_ns_done[0] = g[1] + 1
                            kh_ns_done[1] = g[1] + 1
                    else:
                        vh_group(g[1])
                    clock["pe"] += 8 * MM

                S_list = [(qs, p, jp) for qs in range(4) for p in range(2)
                          for jp in range(8)]
                S_idx = [0]

                def s_ready():
                    if S_idx[0] >= len(S_list):
                        return False
                    qs, p, jp = S_list[S_idx[0]]
                    return (qs, p) in qh_done and kh_ns_done[p] * 4 >= 2 * jp + 2

                def emit_s():
                    score_step(*S_list[S_idx[0]])
                    S_idx[0] += 1
                    if S_idx[0] >= len(S_list):
                        act_free[0] = True
                    clock["pe"] += 4 * MM
                    clock["act"] = max(clock["act"], clock["pe"]) + 2292

                def c_ready():
                    return bool(cq) and 2 * cq[0][2] + 1 < vh_done[0]

                def emit_c():
                    c_entry()
                    clock["pe"] += 2 * MM

                def emit_d():
                    for qs2 in range(4):
                        if norm_done[qs2] == 4 and D_iters[qs2] is not None:
                            if next(D_iters[qs2], _F_done) is _F_done:
                                D_iters[qs2] = None
                                continue
                            clock["pe"] += 2 * MM
                            return True
                    return False

                def d_left():
                    return any(it is not None for it in D_iters)

                while (S_idx[0] < len(S_list) or cq or P_idx[0] < len(P_order)
                       or d_left()):
                    act_lead = clock["act"] - clock["pe"]
                    if s_ready() and act_lead < 4600:
                        emit_s()
                        continue
                    c_floor = 2 if S_idx[0] < 56 else 0
                    if c_ready() and len(cq) > c_floor:
                        emit_c()
                        continue
                    if P_idx[0] < len(P_order) and p_eta() < clock["pe"] + 1200:
                        emit_p()
                        continue
                    if c_ready():
                        emit_c()
                        continue
                    if emit_d():
                        continue
                    if s_ready():
                        emit_s()
                        continue
                    if P_idx[0] < len(P_order):
                        emit_p()
                        continue
                    if cq:
                        c_entry()
                        continue
                    break

    nc.compile()
    return nc


def _get_nc():
    global _NC
    if _NC is None:
        _NC = _build()
    return _NC


def run(inputs, trace=False, trace_cores=None):
    from concourse.bass_utils import run_bass_kernel_spmd

    q = np.asarray(inputs["q"], np.float32)
    k = np.asarray(inputs["k"], np.float32)
    v = np.asarray(inputs["v"], np.float32)
    w_q = np.asarray(inputs["w_q"], np.float32)
    w_k = np.asarray(inputs["w_k"], np.float32)
    w_v = np.asarray(inputs["w_v"], np.float32)
    w_out = np.asarray(inputs["w_out"], np.float32)
    b_q = np.asarray(inputs["b_q"], np.float32)
    b_k = np.asarray(inputs["b_k"], np.float32)
    b_v = np.asarray(inputs["b_v"], np.float32)
    b_out = np.asarray(inputs["b_out"], np.float32)

    import ml_dtypes
    bf16 = ml_dtypes.bfloat16

    def pack_x(x):
        # [S, D] -> transposed, SBUF-layout packed [128, ns*4096+kc*512+c]
        xT = x.T.reshape(8, 128, 4, 512)
        return np.ascontiguousarray(
            xT.transpose(1, 2, 0, 3).reshape(128, 16384).astype(bf16))

    def pack_w(w):
        # [1024, 256] (pre-transposed slice) -> [128, kc*256+c]
        return np.ascontiguousarray(
            w.reshape(8, 128, 256).transpose(1, 0, 2).reshape(128, 2048).astype(bf16))

    xT = {b: {} for b in range(B)}
    for b in range(B):
        xT[b]["qT"] = pack_x(q[b])
        xT[b]["kT"] = pack_x(k[b])
        xT[b]["vT"] = pack_x(v[b])

    in_maps = []
    for c in range(N_CORES):
        b, hq = c // 4, c % 4
        rows = slice(hq * 256, (hq + 1) * 256)
        wo_p = w_out[:, rows].T.reshape(2, 128, 1024)
        in_maps.append({
            "qT": xT[b]["qT"],
            "kT": xT[b]["kT"],
            "vT": xT[b]["vT"],
            "wq": pack_w(w_q[rows, :].T),
            "wk": pack_w(w_k[rows, :].T),
            "wv": pack_w(w_v[rows, :].T),
            "wo": np.ascontiguousarray(
                wo_p.transpose(1, 0, 2).reshape(128, 2048).astype(bf16)),
            "bq": np.ascontiguousarray(b_q[rows].reshape(2, 128).T),
            "bk": np.ascontiguousarray(b_k[rows].reshape(2, 128).T),
            "bv": np.ascontiguousarray(b_v[rows].reshape(1, 256)),
        })

    nc = _get_nc()
    res = run_bass_kernel_spmd(
        nc, in_maps, core_ids=list(range(N_CORES)), trace=trace,
        trace_cores=trace_cores,
    )
    full = np.zeros((B, S, D), np.float32)
    for c in range(N_CORES):
        full[c // 4] += np.asarray(res.results[c]["out"])
    full += b_out.reshape(1, 1, D)
    return full, res.exec_time_ns


def kernel(**inputs):
    return run(inputs, trace=False)[0]


# revision 12
# speedup vs baseline: 1.1393x; 1.1075x over previous
"""Multi-head attention on 8 Trainium2 cores — v2b.

Sharding: core c handles batch b = c // 4 and a quad of 4 heads
(hq = c % 4 -> heads 4*hq .. 4*hq+3) as two head-pairs of 64+64 = 128
partitions. w_q/w_k/w_v split column-wise by head, w_out row-wise;
per-batch partial outputs are summed on host.

v2b structure (vs v1):
  - scores for the even/odd head of a pair issue back-to-back as
    64x128 PE row tiles (0,0)/(64,0) -> they execute concurrently on
    disjoint row groups + separate PSUM banks (2x effective scores).
  - qT/kT/vT fully resident in SBUF; all input DMA is issued up front
    on the sync/gpsimd/vector queues.  The scalar (ACT) queue carries
    only the small weight tensors so exp never queues behind DMA.
  - the qs=0 score steps + exp + C interleave INTO the projection
    phase (kh/vh production) so the ACT engine starts ~14us in
    instead of ~70us.
  - PSUM: scores 2x[128,1024] + C 2x[65,512] + shared proj/D
    2x[128,512] = exactly 8 banks.  Proj/D psum groups are emitted
    atomically so the shared pool never deadlocks a long-lived
    accumulation against fillers.
"""

import numpy as np

B = 2
S = 2048
D = 1024
NH = 16
DH = 64
HEADS_PER_CORE = 4
N_CORES = 8

_NC = None


def _build():
    import concourse.bacc as bacc
    import concourse.tile as tile
    import concourse.mybir as mybir

    fp32 = mybir.dt.float32
    bf16 = mybir.dt.bfloat16
    add = mybir.AluOpType.add
    mult = mybir.AluOpType.mult
    Exp = mybir.ActivationFunctionType.Exp

    nc = bacc.Bacc("TRN2", target_bir_lowering=False)

    # all inputs host-packed to the exact SBUF layout so each is a
    # few large fully-contiguous DMA transfers:
    #   x: [128, ns*4096 + kc*512 + c]  (ns-major blocks of 1MB)
    #   w: [128, kc*256 + c]            (single 512KB transfer)
    qT = nc.dram_tensor("qT", (128, 16384), bf16, kind="ExternalInput")
    kT = nc.dram_tensor("kT", (128, 16384), bf16, kind="ExternalInput")
    vT = nc.dram_tensor("vT", (128, 16384), bf16, kind="ExternalInput")
    wq = nc.dram_tensor("wq", (128, 2048), bf16, kind="ExternalInput")
    wk = nc.dram_tensor("wk", (128, 2048), bf16, kind="ExternalInput")
    wv = nc.dram_tensor("wv", (128, 2048), bf16, kind="ExternalInput")
    wo = nc.dram_tensor("wo", (128, 2048), bf16, kind="ExternalInput")
    bq = nc.dram_tensor("bq", (128, 2), fp32, kind="ExternalInput")
    bk = nc.dram_tensor("bk", (128, 2), fp32, kind="ExternalInput")
    bv = nc.dram_tensor("bv", (1, 256), fp32, kind="ExternalInput")
    out = nc.dram_tensor("out", (S, D), fp32, kind="ExternalOutput")

    with tile.TileContext(nc) as tc:
        with tc.tile_pool(name="persist", bufs=1) as P:
            qhT = [P.tile((128, S), bf16, name=f"qhT{p}") for p in range(2)]
            khT = [P.tile((128, S), bf16, name=f"khT{p}") for p in range(2)]
            vh = [P.tile((128, 16 * 65), bf16, name=f"vh{h}") for h in range(4)]
            outnT = [P.tile((128, S), bf16, name=f"outnT{p}") for p in range(2)]
            wq_sb = P.tile((128, 8 * 256), bf16, name="wq_sb")
            wk_sb = P.tile((128, 8 * 256), bf16, name="wk_sb")
            wv_sb = P.tile((128, 8 * 256), bf16, name="wv_sb")
            wo_t = P.tile((128, 2048), bf16, name="wo_t")
            wo_sb = [wo_t[:, 0:1024], wo_t[:, 1024:2048]]
            bq_sb = P.tile((128, 2), fp32, name="bq_sb")
            bk_sb = P.tile((128, 2), fp32, name="bk_sb")
            bv_row = P.tile((1, 256), fp32, name="bv_row")
            bv_bc = P.tile((128, 256), fp32, name="bv_bc")
            qxt = P.tile((128, 16384), bf16, name="qxt")
            kxt = P.tile((128, 16384), bf16, name="kxt")
            vxt = P.tile((128, 16384), bf16, name="vxt")

            # ones columns for the denominator trick: memset the whole vh
            # tiles to 1.0 on gpsimd (proj overwrites the 64 data columns)
            for h in range(4):
                nc.gpsimd.memset(vh[h][:], 1.0)

            # ---------------- DMA schedule ----------------
            # weights lead the sync queue so they complete before the bulk
            # x blocks behind them; scalar (ACT) queue carries only the two
            # tiny bias transfers and is immediately free for exp.
            nc.sync.dma_start(wq_sb[:], wq[:])
            nc.sync.dma_start(wk_sb[:], wk[:])
            nc.scalar.dma_start(bq_sb[:], bq[:])
            nc.scalar.dma_start(bk_sb[:], bk[:])

            def xblk(eng, dst, srcd, ns):
                eng.dma_start(
                    dst[:, ns * 4096:(ns + 1) * 4096],
                    srcd[:, ns * 4096:(ns + 1) * 4096],
                )

            # need-ordered x in column-half blocks: each ns-block split
            # across both bulk queues so the two halves arrive in parallel.
            def xhalf(eng, dst, srcd, ns, half):
                c0 = ns * 4096 + half * 2048
                eng.dma_start(dst[:, c0:c0 + 2048], srcd[:, c0:c0 + 2048])

            def xpair(dst, srcd, ns):
                xhalf(nc.sync, dst, srcd, ns, 0)
                xhalf(nc.gpsimd, dst, srcd, ns, 1)

            # gpsimd leads with q0/k0 halves while sync still moves weights
            xhalf(nc.gpsimd, qxt, qT, 0, 1)
            xhalf(nc.gpsimd, kxt, kT, 0, 1)
            xhalf(nc.sync, qxt, qT, 0, 0)
            xhalf(nc.sync, kxt, kT, 0, 0)
            xpair(kxt, kT, 1)
            xpair(qxt, qT, 1)
            xpair(kxt, kT, 2)
            nc.gpsimd.dma_start(wv_sb[:], wv[:])
            nc.gpsimd.dma_start(bv_row[:], bv[:])
            nc.gpsimd.partition_broadcast(bv_bc[:], bv_row[:])
            xpair(kxt, kT, 3)
            xpair(vxt, vT, 0)
            xpair(vxt, vT, 1)
            xpair(vxt, vT, 2)
            xpair(vxt, vT, 3)
            xpair(qxt, qT, 2)
            xpair(qxt, qT, 3)
            nc.gpsimd.dma_start(wo_t[:], wo[:])

            # ---------------- compute ----------------
            with tc.tile_pool(name="psS", bufs=2, space="PSUM") as PS, \
                 tc.tile_pool(name="psC", bufs=2, space="PSUM") as PC, \
                 tc.tile_pool(name="psDF", bufs=2, space="PSUM") as PD, \
                 tc.tile_pool(name="expP", bufs=16) as EP, \
                 tc.tile_pool(name="nrm", bufs=2) as NP, \
                 tc.tile_pool(name="outP", bufs=2) as OP:

                # one projection group: 8 accum matmuls + bias add.
                # inject() runs mid-group so a score step can slip in
                # (scores use the PS pool, no PD hazard) and the ACT
                # engine is not starved by long proj bursts.
                def proj_group(xt, w_sb, b_sb, dstT, ns, p, inject=None):
                    ps = PD.tile((128, 512), fp32, name="pps", tag="dps")
                    for kc in range(8):
                        if kc == 4 and inject is not None:
                            inject()
                        nc.tensor.matmul(
                            ps[:],
                            w_sb[:, kc * 256 + p * 128:kc * 256 + (p + 1) * 128],
                            xt[:, ns * 4096 + kc * 512:ns * 4096 + (kc + 1) * 512],
                            start=(kc == 0),
                            stop=(kc == 7),
                        )
                    nc.vector.tensor_scalar_add(
                        dstT[p][:, ns * 512:(ns + 1) * 512], ps[:], b_sb[:, p:p + 1]
                    )

                # one vh group: j-chunk jc for all 4 heads
                def vh_group(jc, inject=None):
                    vh_done[0] = jc + 1
                    ps = PD.tile((128, 512), fp32, name="vps", tag="dps")
                    for kc in range(8):
                        if kc == 4 and inject is not None:
                            inject()
                        nc.tensor.matmul(
                            ps[:, 0:256],
                            vxt[:, (jc // 4) * 4096 + kc * 512 + (jc % 4) * 128:
                                 (jc // 4) * 4096 + kc * 512 + (jc % 4) * 128 + 128],
                            wv_sb[:, kc * 256:(kc + 1) * 256],
                            start=(kc == 0),
                            stop=(kc == 7),
                        )
                    for h in range(4):
                        nc.vector.scalar_tensor_tensor(
                            vh[h][:, jc * 65:jc * 65 + 64],
                            ps[:, h * 64:(h + 1) * 64],
                            1.0,
                            bv_bc[:, h * 64:(h + 1) * 64],
                            op0=mult,
                            op1=add,
                        )

                # ----- scores / exp / C streams -----
                cq = []
                cps_cur = {}
                norm_done = [0] * 4

                act_free = [False]
                Copy = mybir.ActivationFunctionType.Copy

                def normalize(ph, pqs, pcps):
                    p, off = ph // 2, (ph % 2) * 64
                    den = NP.tile((1, 512), fp32, name="den")
                    nc.vector.tensor_scalar_add(den[:], pcps[64:65, :], 0.0)
                    rec = NP.tile((1, 512), fp32, name="rec")
                    nc.vector.reciprocal_approx_fast(rec[:], den[:])
                    rbc = NP.tile((64, 512), fp32, name="rbc")
                    nc.gpsimd.partition_broadcast(rbc[:], rec[:])
                    nc.vector.scalar_tensor_tensor(
                        outnT[p][off:off + 64, pqs * 512:(pqs + 1) * 512],
                        pcps[0:64, :],
                        1.0,
                        rbc[:],
                        op0=mult,
                        op1=mult,
                    )

                def score_step(qs, p, jp):
                    if jp == 0:
                        cps_cur[p] = (
                            PC.tile((65, 512), fp32, name="cps_e", tag="cps"),
                            PC.tile((65, 512), fp32, name="cps_o", tag="cps"),
                        )
                    sps_e = PS.tile((128, 1024), fp32, name="sps_e", tag="sps")
                    sps_o = PS.tile((128, 1024), fp32, name="sps_o", tag="sps")
                    for half in range(2):
                        jc = jp * 2 + half
                        for off, sps in ((0, sps_e), (64, sps_o)):
                            nc.tensor.matmul(
                                sps[:, half * 512:(half + 1) * 512],
                                khT[p][off:off + 64, jc * 128:(jc + 1) * 128],
                                qhT[p][off:off + 64, qs * 512:(qs + 1) * 512],
                                start=True,
                                stop=True,
                            )
                    ex_e = EP.tile((128, 1024), bf16, name="ex_e", tag="ex")
                    nc.scalar.activation(ex_e[:], sps_e[:], Exp, bias=0.0, scale=0.125)
                    ex_o = EP.tile((128, 1024), bf16, name="ex_o", tag="ex")
                    nc.scalar.activation(ex_o[:], sps_o[:], Exp, bias=0.0, scale=0.125)
                    ce, co = cps_cur[p]
                    cq.append((2 * p, qs, jp, ce, ex_e))
                    cq.append((2 * p + 1, qs, jp, co, ex_o))

                S_steps = iter(
                    [(qs, p, jp) for qs in range(4) for p in range(2) for jp in range(8)]
                )

                def run_s(n=1):
                    for _ in range(n):
                        nxt = next(S_steps, None)
                        if nxt is not None:
                            score_step(*nxt)

                def c_entry():
                    ph, pqs, pjp, pcps, pex = cq.pop(0)
                    for half in range(2):
                        pjc = pjp * 2 + half
                        nc.tensor.matmul(
                            pcps[:],
                            vh[ph][:, pjc * 65:(pjc + 1) * 65],
                            pex[:, half * 512:(half + 1) * 512],
                            start=(pjc == 0),
                            stop=(pjc == 15),
                        )
                    if pjp == 7:
                        normalize(ph, pqs, pcps)
                        norm_done[pqs] += 1

                vh_done = [0]  # j-chunks of vh emitted so far

                def run_c(n=1):
                    for _ in range(n):
                        if cq and 2 * cq[0][2] + 1 < vh_done[0]:
                            c_entry()

                # D stage generator: yields once per psum group
                def d_qs(qs):
                    for qq in range(4):
                        qc0 = qs * 512 + qq * 128
                        osb = OP.tile((128, D), fp32, name="osb")
                        for nsd in range(2):
                            dps = PD.tile((128, 512), fp32, name="dps", tag="dps")
                            for p in range(2):
                                nc.tensor.matmul(
                                    dps[:],
                                    outnT[p][:, qc0:qc0 + 128],
                                    wo_sb[p][:, nsd * 512:(nsd + 1) * 512],
                                    start=(p == 0),
                                    stop=(p == 1),
                                )
                            nc.vector.tensor_scalar_add(
                                osb[:, nsd * 512:(nsd + 1) * 512], dps[:], 0.0
                            )
                            yield
                        eng = nc.sync if qq % 2 else nc.gpsimd
                        eng.dma_start(out[qc0:qc0 + 128, :], osb[:])

                D_iters = [d_qs(0), d_qs(1), d_qs(2), d_qs(3)]
                _F_done = object()
                F_qh = iter([(2, 0), (2, 1), (3, 0), (3, 1)])

                def run_f(n=1):
                    for _ in range(n):
                        nxt = next(F_qh, None)
                        if nxt is not None:
                            proj_group(qxt, wq_sb, bq_sb, qhT, *nxt)
                            continue
                        for qs2 in range(4):
                            if norm_done[qs2] == 4 and D_iters[qs2] is not None:
                                if next(D_iters[qs2], _F_done) is _F_done:
                                    D_iters[qs2] = None
                                    continue
                                break

                # ---------------- emission program ----------------
                # Just-in-time scheduler: keep ACT fed with score steps,
                # drain C close behind, pace proj/vh groups by their DMA
                # arrival ETAs so a stalled matmul never head-of-line
                # blocks ready work on the in-order PE queue.
                MM = 250.0
                clock = {"pe": 10000.0, "act": 14000.0}

                K_ETA = [14500, 18000, 21500, 26500]
                Q_ETA = [14500, 21500, 41000, 44000]
                V_ETA = [29000, 31500, 34000, 36500]

                P_order = (
                    [("q", 0, 0), ("q", 0, 1)]
                    + [("k", ns, p) for ns in range(4) for p in range(2)]
                    + [("q", 1, 0), ("q", 1, 1)]
                    + [("v", jc) for jc in range(16)]
                    + [("q", 2, 0), ("q", 2, 1), ("q", 3, 0), ("q", 3, 1)]
                )
                P_idx = [0]
                qh_done = set()
                kh_ns_done = [0, 0]

                def p_eta():
                    g = P_order[P_idx[0]]
                    if g[0] == "k":
                        return K_ETA[g[1]]
                    if g[0] == "q":
                        return Q_ETA[g[1]]
                    return V_ETA[g[1] // 4]

                def maybe_s():
                    if s_ready() and clock["act"] - clock["pe"] < 2400:
                        emit_s()

                def emit_p():
                    g = P_order[P_idx[0]]
                    P_idx[0] += 1
                    if g[0] == "q":
                        proj_group(qxt, wq_sb, bq_sb, qhT, g[1], g[2])
                        qh_done.add((g[1], g[2]))
                    elif g[0] == "k":
                        proj_group(kxt, wk_sb, bk_sb, khT, g[1], g[2])
                        if g[2] == 1:
                            kh_ns_done[0] = g[1] + 1
                            kh_ns_done[1] = g[1] + 1
                    else:
                        vh_group(g[1])
                    clock["pe"] += 8 * MM

                S_list = [(qs, p, jp) for qs in range(4) for p in range(2)
                          for jp in range(8)]
                S_idx = [0]

                def s_ready():
                    if S_idx[0] >= len(S_list):
                        return False
                    qs, p, jp = S_list[S_idx[0]]
                    return (qs, p) in qh_done and kh_ns_done[p] * 4 >= 2 * jp + 2

                def emit_s():
                    score_step(*S_list[S_idx[0]])
                    S_idx[0] += 1
                    if S_idx[0] >= len(S_list):
                        act_free[0] = True
                    clock["pe"] += 4 * MM
                    clock["act"] = max(clock["act"], clock["pe"]) + 2292

                def c_ready():
                    return bool(cq) and 2 * cq[0][2] + 1 < vh_done[0]

                def emit_c():
                    c_entry()
                    clock["pe"] += 2 * MM

                def emit_d():
                    for qs2 in range(4):
                        if norm_done[qs2] == 4 and D_iters[qs2] is not None:
                            if next(D_iters[qs2], _F_done) is _F_done:
                                D_iters[qs2] = None
                                continue
                            clock["pe"] += 2 * MM
                            return True
                    return False

                def d_left():
                    return any(it is not None for it in D_iters)

                while (S_idx[0] < len(S_list) or cq or P_idx[0] < len(P_order)
                       or d_left()):
                    act_lead = clock["act"] - clock["pe"]
                    if s_ready() and act_lead < 4600:
                        emit_s()
                        continue
                    c_floor = 2 if S_idx[0] < 56 else 0
                    if c_ready() and len(cq) > c_floor:
                        emit_c()
                        continue
                    if P_idx[0] < len(P_order) and p_eta() < clock["pe"] + 1200:
                        emit_p()
                        continue
                    if c_ready():
                        emit_c()
                        continue
                    if emit_d():
                        continue
                    if s_ready():
                        emit_s()
                        continue
                    if P_idx[0] < len(P_order):
                        emit_p()
                        continue
                    if cq:
                        c_entry()
                        continue
                    break

    nc.compile()
    return nc


def _get_nc():
    global _NC
    if _NC is None:
        _NC = _build()
    return _NC


def run(inputs, trace=False, trace_cores=None):
    from concourse.bass_utils import run_bass_kernel_spmd

    q = np.asarray(inputs["q"], np.float32)
    k = np.asarray(inputs["k"], np.float32)
    v = np.asarray(inputs["v"], np.float32)
    w_q = np.asarray(inputs["w_q"], np.float32)
    w_k = np.asarray(inputs["w_k"], np.float32)
    w_v = np.asarray(inputs["w_v"], np.float32)
    w_out = np.asarray(inputs["w_out"], np.float32)
    b_q = np.asarray(inputs["b_q"], np.float32)
    b_k = np.asarray(inputs["b_k"], np.float32)
    b_v = np.asarray(inputs["b_v"], np.float32)
    b_out = np.asarray(inputs["b_out"], np.float32)

    import ml_dtypes
    bf16 = ml_dtypes.bfloat16

    def pack_x(x):
        # [S, D] -> transposed, SBUF-layout packed [128, ns*4096+kc*512+c]
        xT = x.T.reshape(8, 128, 4, 512)
        return np.ascontiguousarray(
            xT.transpose(1, 2, 0, 3).reshape(128, 16384).astype(bf16))

    def pack_w(w):
        # [1024, 256] (pre-transposed slice) -> [128, kc*256+c]
        return np.ascontiguousarray(
            w.reshape(8, 128, 256).transpose(1, 0, 2).reshape(128, 2048).astype(bf16))

    xT = {b: {} for b in range(B)}
    for b in range(B):
        xT[b]["qT"] = pack_x(q[b])
        xT[b]["kT"] = pack_x(k[b])
        xT[b]["vT"] = pack_x(v[b])

    in_maps = []
    for c in range(N_CORES):
        b, hq = c // 4, c % 4
        rows = slice(hq * 256, (hq + 1) * 256)
        wo_p = w_out[:, rows].T.reshape(2, 128, 1024)
        in_maps.append({
            "qT": xT[b]["qT"],
            "kT": xT[b]["kT"],
            "vT": xT[b]["vT"],
            "wq": pack_w(w_q[rows, :].T),
            "wk": pack_w(w_k[rows, :].T),
            "wv": pack_w(w_v[rows, :].T),
            "wo": np.ascontiguousarray(
                wo_p.transpose(1, 0, 2).reshape(128, 2048).astype(bf16)),
            "bq": np.ascontiguousarray(b_q[rows].reshape(2, 128).T),
            "bk": np.ascontiguousarray(b_k[rows].reshape(2, 128).T),
            "bv": np.ascontiguousarray(b_v[rows].reshape(1, 256)),
        })

    nc = _get_nc()
    res = run_bass_kernel_spmd(
        nc, in_maps, core_ids=list(range(N_CORES)), trace=trace,
        trace_cores=trace_cores,
    )
    full = np.zeros((B, S, D), np.float32)
    for c in range(N_CORES):
        full[c // 4] += np.asarray(res.results[c]["out"])
    full += b_out.reshape(1, 1, D)
    return full, res.exec_time_ns


def kernel(**inputs):
    return run(inputs, trace=False)[0]
